# revision 1
# baseline (speedup 1.0000x reference)
"""Trainium2 Bass kernel: fused embedding gather-concat.

out[r] = concat(proc_pos[process_ids[r]], locs_sp[location_ids[r]])   r in [0, 8M)

Sharding: rows data-parallel across 8 NeuronCores (1M rows each, padded to a
batch multiple); proc_pos / locs_sp tables replicated to every core.

Raw-bass software pipeline per core, batches of 128*K rows:
  SP   : DMA the two int32 index tiles [128, K] into SBUF slots
  Pool : per-column indirect-DMA gathers — each instruction gathers 128 rows
         (one int32 index per partition; the only indirect addressing mode the
         walrus lowering supports), writing proc (32B) and loc (12B) blocks
         directly interleaved into the assembled out tile
  ACT  : one contiguous [128, K*11] store per batch
Cross-engine flow control via per-slot DMA-completion semaphores over a
`depth`-slot ring buffer (per-slot counters keep waits unambiguous under
out-of-order DMA completion).

Perf: bound at ~22.4 ms/core by the ~1.1 us fixed SWDGE descriptor-gen cost
of each 128-row DMA_INDIRECT instruction (15,680 per core). Measured-dead
ends: (a) indirect_dma_start is gpsimd-only, so the stream cannot be spread
across engine queues; (b) round-robining instruction.queue over the 4
declared SWDGE queues (num_swdge_queues=4, qPoolDynamic{,1,2,3} — kept here,
harmless) measured identical 22.39 ms, i.e. no per-queue parallelism
materializes for walrus-lowered DMA_INDIRECT; (c) dma_gather (the raw-ISA
multi-queue gather) asserts int16 indices + 256B-multiple rows, so it cannot
address the 500K x 12B loc table; (d) multi-index offset tiles silently
degrade to a per-partition contiguous span fetch, so rows-per-instruction
cannot be raised. Remaining known lever: hand-emitting DMA_INDIRECT1D ISA
structs with up to 4096 uint32 indices per instruction.
"""

from contextlib import ExitStack

import numpy as np

import concourse.bass as bass
import concourse.mybir as mybir
from concourse.bass_utils import run_bass_kernel_spmd

N_CORES = 8
NAUG = 8_000_000
PER_CORE = NAUG // N_CORES  # 1,000,000

NUM_PROCS = 16
PROC_DIM = 8
NUM_LOCS = 500_000
SPATIAL_DIM = 3
OUT_DIM = PROC_DIM + SPATIAL_DIM  # 11

# The BIR verifier requires an indirect-DMA source AP to span at least the
# gathered destination size, so the 16-row proc table is padded with zeros.
PROC_PAD = 65536

P = 128  # SBUF partitions


def build_nc(n_rows, k, num_locs=NUM_LOCS, depth=4):
    """Build the single-core Bass module. n_rows must be divisible by 128*k."""
    r = P * k  # rows per batch
    nb = n_rows // r
    assert nb * r == n_rows
    assert depth >= 2

    nc = bass.Bass(num_swdge_queues=4)
    proc_pos = nc.declare_dram_parameter(
        "proc_pos", [PROC_PAD, PROC_DIM], mybir.dt.float32, isOutput=False
    )
    locs_sp = nc.declare_dram_parameter(
        "locs_sp", [num_locs, SPATIAL_DIM], mybir.dt.float32, isOutput=False
    )
    process_ids = nc.declare_dram_parameter(
        "process_ids", [n_rows], mybir.dt.int32, isOutput=False
    )
    location_ids = nc.declare_dram_parameter(
        "location_ids", [n_rows], mybir.dt.int32, isOutput=False
    )
    out = nc.declare_dram_parameter(
        "out", [n_rows, OUT_DIM], mybir.dt.float32, isOutput=True
    )

    # Row assignment within a batch: row index = base + p*k + t for partition p,
    # column t. Index tiles load as [128, k] contiguous; gather t uses column t
    # (one index per partition, 128 rows per indirect DMA — the only indirect
    # addressing mode the toolchain supports); stores stay fully contiguous.
    pids_v = process_ids.rearrange("(b p k) -> b p k", b=nb, p=P)
    lids_v = location_ids.rearrange("(b p k) -> b p k", b=nb, p=P)
    out_v = out.rearrange("(b p k) d -> b p (k d)", b=nb, p=P)

    kk = k * OUT_DIM

    def uses(s):
        """How many batches land in slot s."""
        return (nb - s + depth - 1) // depth

    with ExitStack() as ctx:
        # Per-slot semaphores: slot reuse is strictly serialized by the waits,
        # so each per-slot count is unambiguous even though DMA completions
        # across different slots/batches are unordered.
        sem_ld = [ctx.enter_context(nc.semaphore(f"sem_ld{s}")) for s in range(depth)]
        sem_g = [ctx.enter_context(nc.semaphore(f"sem_g{s}")) for s in range(depth)]
        sem_st = [ctx.enter_context(nc.semaphore(f"sem_st{s}")) for s in range(depth)]
        pid_buf = ctx.enter_context(
            nc.sbuf_tensor("pid_buf", [P, depth * k], mybir.dt.int32)
        )
        lid_buf = ctx.enter_context(
            nc.sbuf_tensor("lid_buf", [P, depth * k], mybir.dt.int32)
        )
        out_buf = ctx.enter_context(
            nc.sbuf_tensor("out_buf", [P, depth * kk], mybir.dt.float32)
        )
        block = ctx.enter_context(nc.Block())
        gpb = 2 * k  # gather instructions per batch
        # Spread the SWDGE instruction stream across the 4 hardware queues
        # (the ~1.1us/instruction descriptor-gen cost is per-queue serial).
        QNAMES = ["qPoolDynamic", "qPoolDynamic1", "qPoolDynamic2", "qPoolDynamic3"]

        @block.sync
        def _(sp):
            for b in range(nb):
                s, u = b % depth, b // depth
                if u >= 1:
                    # index slot free once its previous gathers are done
                    sp.wait_ge(sem_g[s], 16 * gpb * u)
                sp.dma_start(
                    out=pid_buf[:, s * k : (s + 1) * k], in_=pids_v[b]
                ).then_inc(sem_ld[s], 16)
                sp.dma_start(
                    out=lid_buf[:, s * k : (s + 1) * k], in_=lids_v[b]
                ).then_inc(sem_ld[s], 16)
            for s in range(depth):
                sp.wait_ge(sem_ld[s], 32 * uses(s))

        @block.gpsimd
        def _(gp):
            for b in range(nb):
                s, u = b % depth, b // depth
                gp.wait_ge(sem_ld[s], 32 * (u + 1))
                if u >= 1:
                    # out slot free once its previous store is done
                    gp.wait_ge(sem_st[s], 16 * u)
                out3 = out_buf[:, s * kk : (s + 1) * kk].rearrange(
                    "p (k d) -> p k d", d=OUT_DIM
                )
                for t in range(k):
                    gi = gp.indirect_dma_start(
                        out=out3[:, t, 0:PROC_DIM],
                        out_offset=None,
                        in_=proc_pos[:],
                        in_offset=bass.IndirectOffsetOnAxis(
                            ap=pid_buf[:, s * k + t : s * k + t + 1], axis=0
                        ),
                    ).then_inc(sem_g[s], 16)
                    gi.ins.queue = QNAMES[(2 * t) % 4]
                    gi = gp.indirect_dma_start(
                        out=out3[:, t, PROC_DIM:OUT_DIM],
                        out_offset=None,
                        in_=locs_sp[:],
                        in_offset=bass.IndirectOffsetOnAxis(
                            ap=lid_buf[:, s * k + t : s * k + t + 1], axis=0
                        ),
                    ).then_inc(sem_g[s], 16)
                    gi.ins.queue = QNAMES[(2 * t + 1) % 4]
            for s in range(depth):
                gp.wait_ge(sem_g[s], 16 * gpb * uses(s))

        @block.scalar
        def _(act):
            for b in range(nb):
                s, u = b % depth, b // depth
                act.wait_ge(sem_g[s], 16 * gpb * (u + 1))
                act.dma_start(
                    out=out_v[b], in_=out_buf[:, s * kk : (s + 1) * kk]
                ).then_inc(sem_st[s], 16)
            for s in range(depth):
                act.wait_ge(sem_st[s], 16 * uses(s))

    return nc


# Full-size batch geometry: 20 batches of 128x392 = 50,176 rows -> 1,003,520
# rows per core (0.35% padding over the 1M real rows).
K = 392
R = P * K
NB = -(-PER_CORE // R)
N_PAD = NB * R

_nc_cache = {}

# test.py reads this for exec_time_ns / trace info after a traced run.
_last_results = None


def _get_nc():
    key = (N_PAD, K)
    if key not in _nc_cache:
        _nc_cache[key] = build_nc(N_PAD, K)
    return _nc_cache[key]


def kernel(proc_pos, locs_sp, process_ids, location_ids):
    global _last_results
    proc_pos = np.ascontiguousarray(np.asarray(proc_pos, dtype=np.float32))
    locs_sp = np.ascontiguousarray(np.asarray(locs_sp, dtype=np.float32))
    proc_pad = np.zeros((PROC_PAD, PROC_DIM), np.float32)
    proc_pad[: proc_pos.shape[0]] = proc_pos
    pids = np.asarray(process_ids).astype(np.int32, copy=False)
    lids = np.asarray(location_ids).astype(np.int32, copy=False)

    nc = _get_nc()
    in_maps = []
    for c in range(N_CORES):
        lo, hi = c * PER_CORE, (c + 1) * PER_CORE
        pid_c = np.zeros(N_PAD, np.int32)
        lid_c = np.zeros(N_PAD, np.int32)
        pid_c[:PER_CORE] = pids[lo:hi]
        lid_c[:PER_CORE] = lids[lo:hi]
        in_maps.append(
            {
                "proc_pos": proc_pad,
                "locs_sp": locs_sp,
                "process_ids": pid_c,
                "location_ids": lid_c,
            }
        )

    res = run_bass_kernel_spmd(nc, in_maps, list(range(N_CORES)))
    _last_results = res
    out = np.concatenate([r["out"][:PER_CORE] for r in res.results], axis=0)
    return out

